# revision 68
# baseline (speedup 1.0000x reference)
"""Multi-head self-attention (B=2, S=2048, D=1024, H=16) on 8 TRN2 NeuronCores.

Tensor-parallel over heads: each core owns 2 heads. Accepts FULL inputs,
returns FULL output. Host pre-tiles x^T and slices per-head weights; each
core computes qkv -> per-head LayerNorm -> attention -> transposed partial
projection (over its 128 embed dims); host sums the 8 partials, transposes,
and adds the projection bias.

Perf notes (vs the first working version):
- softmax denominator: vO carries 64 ones-columns so the AV matmul
  broadcasts the denominator across partitions 64:128 for free; one DVE
  divide per head replaces the Ln -> PE-broadcast -> Exp(-x) -> mult chain.
- ACT only ever runs Exp/Ln/Copy (rstd = exp(-0.5*ln(var+eps))), all in
  one activation-table set -> no ACT_TABLE_LOAD switches.
- projection runs transposed (wp stationary, aT moving) and its PSUM is
  DMA'd straight to DRAM; bias is added on the host after the 8-way sum.
- PE instruction stream is software-pipelined (scores a step ahead of AV,
  prev-qc projection matmuls fill the qc-boundary epilogue gap).
"""

import os
import sys

import numpy as np

for _p in ("/opt/trn_rl_repo", "/root/.axon_site/_ro/trn_rl_repo"):
    if os.path.isdir(_p) and _p not in sys.path:
        sys.path.insert(0, _p)
        break

import concourse.bass as bass  # noqa: E402
import concourse.bacc as bacc  # noqa: E402
import concourse.tile as tile  # noqa: E402
from concourse import mybir  # noqa: E402
from concourse.bass_utils import run_bass_kernel_spmd  # noqa: E402

F32 = mybir.dt.float32
BF16 = mybir.dt.bfloat16
AF = mybir.ActivationFunctionType
ALU = mybir.AluOpType

NCORES = 8
D = 1024
H = 16
HD = 64
HPC = H // NCORES          # heads per core = 2
DPC = HPC * HD             # embed dims per core = 128
EPS = 1e-5


def build_nc(B, S, affine):
    """Build the SPMD Bass program for one core (same program, 8 cores)."""
    T = B * S                      # total token columns
    NTB = T // 128                 # 128-token blocks (32)
    NCH = T // 512                 # 512-token chunks (8)
    QC = S // 512                  # q-chunks per batch (4)
    KB = S // 128                  # k-blocks per batch (16)
    KCH = D // 128                 # contraction chunks (8)
    SCALE = 1.0 / np.sqrt(HD)

    nc = bacc.Bacc(
        "TRN2",
        target_bir_lowering=False,
        debug=False,
        enable_asserts=True,
        num_devices=NCORES,
    )

    # host-pretiled x^T: [128, NCH, KCH, 512] (p, chunk, k, tok)
    xT = nc.dram_tensor("xTt", [128, NCH, KCH, 512], BF16, kind="ExternalInput").ap()
    wq = nc.dram_tensor("wt_qkv", [128, KCH, 3 * DPC], BF16, kind="ExternalInput").ap()
    bq = nc.dram_tensor("b_qkv_s", [1, 3 * DPC], BF16, kind="ExternalInput").ap()
    wp = nc.dram_tensor("wt_proj", [DPC, D], BF16, kind="ExternalInput").ap()
    onesb = nc.dram_tensor("c_onesb", [1, 128], BF16, kind="ExternalInput").ap()
    eye = nc.dram_tensor("c_eye", [128, 128], BF16, kind="ExternalInput").ap()
    if affine:
        gb = nc.dram_tensor("c_gb", [128, 4, HD], F32, kind="ExternalInput").ap()
    # transposed partial projection output: [D, T] (bf16 partials; host sums)
    outp = nc.dram_tensor("outp", [D, T], BF16, kind="ExternalOutput").ap()

    from contextlib import ExitStack

    with tile.TileContext(nc) as tc, ExitStack() as stack:
        const = stack.enter_context(tc.tile_pool(name="const", bufs=1))
        persist = stack.enter_context(tc.tile_pool(name="persist", bufs=1))

        # startup DMAs spread across queue rings so the first matmul's
        # prerequisites (bq, onesb, wq k0, xt k0) land as early as possible
        bq_sb = const.tile([1, 3 * DPC], BF16, tag="bq")
        nc.gpsimd.dma_start(out=bq_sb, in_=bq)
        onesb_sb = const.tile([1, 128], BF16, tag="onesb")
        nc.gpsimd.dma_start(out=onesb_sb, in_=onesb)
        wq_sb = const.tile([128, KCH, 3 * DPC], BF16, tag="wq")
        for _k in range(KCH):
            nc.scalar.dma_start(out=wq_sb[:, _k, :], in_=wq[:, _k, :])
        wp_sb = const.tile([DPC, D], BF16, tag="wp")
        nc.gpsimd.dma_start(out=wp_sb, in_=wp)
        eye_sb = const.tile([128, 128], BF16, tag="eye")
        nc.gpsimd.dma_start(out=eye_sb, in_=eye)
        eps_sb = const.tile([128, 1], F32, tag="eps")
        nc.vector.memset(eps_sb, EPS)
        # PE p-state warmup: dummy matmuls (fed by memset, no DMA wait) ramp
        # the clock toward 2.4 GHz while the first weight DMAs are in flight
        warm = const.tile([128, 384], BF16, tag="warm")
        nc.vector.memset(warm, 0.0)

        if affine:
            gb_sb = const.tile([128, 4, HD], F32, tag="gb")
            nc.sync.dma_start(out=gb_sb, in_=gb)

        # persistent intermediates
        qT = persist.tile([128, T], BF16, tag="qT")     # [2h*64, tok] LN'd q^T
        kT = persist.tile([128, T], BF16, tag="kT")
        # 64 ones-cols + v per head: AV matmul then yields softmax denom
        # broadcast over partitions 0:64 at zero extra PE cost (ones first so
        # the denominator lands at partition base 0, which the custom-DVE
        # reciprocal requires)
        vO = persist.tile([128, HPC, NTB, 128], BF16, tag="vO")
        aT = persist.tile([128, T], BF16, tag="aT")     # attention out^T
        nc.gpsimd.memset(vO[:, :, :, 0:HD], 1.0)

        # stage1 outlives phase 1: the last blocks' transposes are deferred
        # into phase 2 (SBUF-only pool, no PSUM cost)
        stage1 = stack.enter_context(tc.tile_pool(name="stage1", bufs=6))
        pending = []  # (qn, kn, tb)

        # ---------------- Phase 1: qkv + LayerNorm + transpose ----------
        with (
            tc.tile_pool(name="xt", bufs=2) as xt_pool,
            tc.tile_pool(name="qkv_ps", bufs=6, space="PSUM") as qkv_ps,
            tc.tile_pool(name="t_ps", bufs=2, space="PSUM") as t_ps,
            tc.tile_pool(name="stats", bufs=6) as stats_pool,
        ):
            # software pipeline: transposes/copies of block tb are emitted
            # two blocks later in the matmul stream so the PE never waits on
            # the LayerNorm chain.
            def flush_one():
                qn, kn, tb = pending.pop(0)
                tp = t_ps.tile([128, 256], BF16, tag="tp")
                nc.tensor.transpose(tp[:, 0:128], qn, eye_sb)
                nc.tensor.transpose(tp[:, 128:256], kn, eye_sb)
                ts = slice(tb * 128, (tb + 1) * 128)
                nc.vector.tensor_copy(out=qT[:, ts], in_=tp[:, 0:128])
                nc.scalar.copy(out=kT[:, ts], in_=tp[:, 128:256])

            # PE warmup burst before the first real matmul's inputs arrive
            warm_ps = qkv_ps.tile([128, 3 * DPC], F32, tag="ps")
            for _w in range(6):
                nc.tensor.matmul(
                    warm_ps,
                    lhsT=warm[:, 0:128],
                    rhs=warm,
                    start=(_w == 0),
                    stop=(_w == 5),
                )

            # stage-B of the LN chain runs one block behind stage-A so no
            # engine ever waits on a fresh cross-engine result
            chain = []  # (qk, ps, mv, tb)

            def ln_finish():
                qk, ps, mv, tb = chain.pop(0)
                rstd = stats_pool.tile([128, 4], F32, tag="rstd")
                nc.scalar.activation(
                    out=rstd, in_=mv[:, :, 1], func=AF.Sqrt, bias=eps_sb
                )
                nc.vector.reciprocal(out=rstd, in_=rstd)
                # nmr = -mean*rstd lets ACT apply LN as x*rstd + nmr
                nmr = stats_pool.tile([128, 4], F32, tag="nmr")
                nc.vector.scalar_tensor_tensor(
                    out=nmr, in0=mv[:, :, 0], scalar=-1.0, in1=rstd,
                    op0=ALU.mult, op1=ALU.mult,
                )
                qn = stage1.tile([128, 128], BF16, tag="qn")
                kn = stage1.tile([128, 128], BF16, tag="kn")
                for g in range(4):
                    dst = qn if g < 2 else kn
                    dsl = dst[:, (g % 2) * HD : (g % 2 + 1) * HD]
                    if g < 2:
                        nc.vector.tensor_scalar(
                            out=dsl,
                            in0=qk[:, g, :],
                            scalar1=mv[:, g, 0:1],
                            scalar2=rstd[:, g : g + 1],
                            op0=ALU.subtract,
                            op1=ALU.mult,
                        )
                    else:
                        nc.scalar.activation(
                            out=dsl,
                            in_=qk[:, g, :],
                            func=AF.Identity,
                            scale=rstd[:, g : g + 1],
                            bias=nmr[:, g : g + 1],
                        )
                    if affine:
                        nc.vector.tensor_mul(dsl, dsl, gb_sb[:, 2 * (g // 2), :])
                        nc.vector.tensor_add(
                            dsl, dsl, gb_sb[:, 2 * (g // 2) + 1, :]
                        )
                # v into vO (ones-cols already memset)
                nc.scalar.copy(
                    out=vO[:, 0, tb, HD:128],
                    in_=ps[:, 2 * DPC : 2 * DPC + HD],
                )
                nc.scalar.copy(
                    out=vO[:, 1, tb, HD:128],
                    in_=ps[:, 2 * DPC + HD : 2 * DPC + 2 * HD],
                )
                pending.append((qn, kn, tb))

            for n in range(NCH):
                xt = xt_pool.tile([128, KCH, 512], BF16, tag="xt")
                if n == 0:
                    # split the first chunk per k so the first matmuls can
                    # start after 1/8 of the transfer
                    for k in range(KCH):
                        nc.sync.dma_start(out=xt[:, k, :], in_=xT[:, 0, k, :])
                else:
                    nc.sync.dma_start(out=xt, in_=xT[:, n])
                for tbl in range(4):
                    tb = n * 4 + tbl
                    ps = qkv_ps.tile([128, 3 * DPC], F32, tag="ps")
                    # bias first on even blocks, last on odd blocks: the K=1
                    # bias matmuls of adjacent blocks pair up so the PE pays
                    # one K=1<->K=128 transition per block instead of two
                    bias_first = tb % 2 == 0
                    if bias_first:
                        nc.tensor.matmul(
                            ps, lhsT=onesb_sb, rhs=bq_sb, start=True, stop=False
                        )
                    for k in range(KCH):
                        if k == 4 and len(pending) >= 3:
                            flush_one()  # transposes, 3+ blocks behind
                        nc.tensor.matmul(
                            ps,
                            lhsT=xt[:, k, tbl * 128 : (tbl + 1) * 128],
                            rhs=wq_sb[:, k, :],
                            start=(not bias_first and k == 0),
                            stop=(bias_first and k == KCH - 1),
                        )
                    if not bias_first:
                        nc.tensor.matmul(
                            ps, lhsT=onesb_sb, rhs=bq_sb, start=False, stop=True
                        )
                    # LayerNorm stats over each head's 64 dims of q and k
                    qk = ps[:, 0 : 2 * DPC].rearrange("p (g d) -> p g d", d=HD)
                    st = stats_pool.tile([128, 4, 6], F32, tag="st")
                    mv = stats_pool.tile([128, 4, 2], F32, tag="mv")
                    for g in range(4):
                        nc.vector.bn_stats(out=st[:, g, :], in_=qk[:, g, :])
                        nc.vector.bn_aggr(out=mv[:, g, :], in_=st[:, g, :])
                    chain.append((qk, ps, mv, tb))
                    if len(chain) >= 2:
                        ln_finish()  # block tb-1's sqrt/apply/v
            while chain:
                ln_finish()
            # leftover `pending` transposes are deferred into phase 2: the
            # first scores don't need them, and they fill the exp warmup

        # ---------------- Phase 2: attention + fused projection ---------
        with (
            tc.tile_pool(name="sc_ps", bufs=2, space="PSUM") as sc_ps,
            tc.tile_pool(name="o_ps", bufs=3, space="PSUM") as o_ps,
            tc.tile_pool(name="p_ps", bufs=1, space="PSUM") as p_ps,
            tc.tile_pool(name="exps", bufs=4) as exps,
            tc.tile_pool(name="ostage", bufs=3) as ostage,
        ):
            # proj matmuls for q-chunk (b, qc) are emitted inside the next
            # q-chunk's kb loop (aT is ready by then)
            proj_ctr = [0]

            def emit_proj_one(b, qc, nb, drain=False):
                pcols = slice(b * S + qc * 512, b * S + (qc + 1) * 512)
                alt = proj_ctr[0] % 2 == 0
                pps = p_ps.tile([128, 512], F32, tag="pps", name="pps")
                nc.tensor.matmul(
                    pps,
                    lhsT=wp_sb[:, nb * 128 : (nb + 1) * 128],
                    rhs=aT[:, pcols],
                    start=True,
                    stop=True,
                )
                ob = ostage.tile([128, 512], BF16, tag="ob")
                if drain and alt:
                    # ACT is idle after the last exp: split the evictions
                    nc.scalar.copy(out=ob, in_=pps)
                else:
                    nc.vector.tensor_copy(out=ob, in_=pps)
                dq = nc.gpsimd if alt else nc.sync
                proj_ctr[0] += 1
                dq.dma_start(
                    out=outp[nb * 128 : (nb + 1) * 128, pcols], in_=ob
                )

            def emit_scores(b, qc, kb):
                cols = slice(b * S + qc * 512, b * S + (qc + 1) * 512)
                gkb = b * KB + kb
                ks = slice(gkb * 128, (gkb + 1) * 128)
                scp = sc_ps.tile([128, HPC, 512], F32, tag="s", name="scp")
                for h in range(HPC):
                    hp = slice(h * HD, (h + 1) * HD)
                    nc.tensor.matmul(
                        scp[:, h, :],
                        lhsT=kT[hp, ks],
                        rhs=qT[hp, cols],
                        start=True,
                        stop=True,
                    )
                ex = exps.tile([128, HPC, 512], BF16, tag="ex", name="ex")
                nc.scalar.activation(out=ex, in_=scp, func=AF.Exp, scale=SCALE)
                return ex

            proj_jobs = []  # flat (b, qc, nb) list, aT already readable

            # one flat software pipeline over every (b, qc, kb) step: scores
            # run two steps ahead of AV, across q-chunk boundaries, so
            # neither the PE nor ACT ever waits at a boundary
            steps = [
                (b, qc, kb)
                for b in range(B)
                for qc in range(QC)
                for kb in range(KB)
            ]
            exq = [
                emit_scores(*steps[0]),
                emit_scores(*steps[1]),
                emit_scores(*steps[2]),
            ]
            # phase-1 leftovers: last blocks' transposes, hidden under the
            # first exps (their kT/qT regions aren't read until batch 1)
            while pending:
                qn, kn, tb = pending.pop(0)
                tp = p_ps.tile([128, 512], F32, tag="pps", name="tpd")
                tpb = tp.bitcast(BF16)[:, 0:256]
                nc.tensor.transpose(tpb[:, 0:128], qn, eye_sb)
                nc.tensor.transpose(tpb[:, 128:256], kn, eye_sb)
                ts2 = slice(tb * 128, (tb + 1) * 128)
                nc.vector.tensor_copy(out=qT[:, ts2], in_=tpb[:, 0:128])
                nc.vector.tensor_copy(out=kT[:, ts2], in_=tpb[:, 128:256])
            oom = None
            for i, (b, qc, kb) in enumerate(steps):
                cols = slice(b * S + qc * 512, b * S + (qc + 1) * 512)
                if kb == 0:
                    # per-head oom tiles from a 3-deep pool: this q-chunk's
                    # first AV doesn't wait for the previous one's drain
                    oom = [
                        o_ps.tile([128, 512], F32, tag="o", name=f"oom{h}")
                        for h in range(HPC)
                    ]
                if i + 3 < len(steps):
                    exq.append(emit_scores(*steps[i + 3]))
                # AV for kb
                cur_ex = exq.pop(0)
                gkb = b * KB + kb
                for h in range(HPC):
                    nc.tensor.matmul(
                        oom[h],
                        lhsT=vO[:, h, gkb, :],
                        rhs=cur_ex[:, h, :],
                        start=(kb == 0),
                        stop=(kb == KB - 1),
                    )
                # prev-qc projection matmuls keep the PE dense while this
                # q-chunk's exps run; start at kb=6 so their DVE evictions
                # don't queue behind the previous epilogue's DVE chain
                if proj_jobs and 6 <= kb < 14:
                    emit_proj_one(*proj_jobs.pop(0))
                if kb == KB - 1:
                    # epilogue: denominator is broadcast over rows 0:64 of
                    # oom; stage it for the custom-DVE reciprocal, multiply
                    # the AV half (rows 64:128) straight from PSUM. The
                    # 3-deep oom pool hides this chain from the next q-chunk.
                    last = i == len(steps) - 1
                    dnms = []
                    for h in range(HPC):
                        dnm = ostage.tile([HD, 512], F32, tag=f"dnm{h}")
                        # on the very last chunk ACT stages head 1's denom in
                        # parallel to shorten the serial DVE tail chain
                        if last and h == 1:
                            nc.scalar.copy(out=dnm, in_=oom[h][0:HD, :])
                        else:
                            nc.vector.tensor_copy(out=dnm, in_=oom[h][0:HD, :])
                        dnms.append(dnm)
                    for h in range(HPC):
                        rcp = ostage.tile([HD, 512], F32, tag=f"rcp{h}")
                        nc.vector.reciprocal_approx_fast(out=rcp, in_=dnms[h])
                        nc.vector.tensor_mul(
                            aT[h * HD : (h + 1) * HD, cols],
                            oom[h][HD:128, :],
                            rcp,
                        )
                    proj_jobs.extend((b, qc, nb) for nb in range(KCH))
            # drain remaining projections
            for job in proj_jobs:
                emit_proj_one(*job, drain=True)

    nc.compile()
    return nc


def make_in_maps(x, w_qkv, b_qkv, w_proj, b_proj, q_gamma, q_beta, k_gamma, k_beta,
                 affine):
    import ml_dtypes
    bf = ml_dtypes.bfloat16
    B, S, _ = x.shape
    T = B * S
    NCH = T // 512
    KCH = D // 128
    # x^T tiled: X[p, n, c, t] = x2[c*128+p, n*512+t] where x2 = x.reshape(T,D).T
    x2 = np.ascontiguousarray(x.reshape(T, D).T.astype(bf))       # [D, T]
    Xt = np.ascontiguousarray(
        x2.reshape(KCH, 128, NCH, 512).transpose(1, 2, 0, 3)
    )  # [128, NCH, KCH, 512]
    eye = np.eye(128, dtype=np.float32)
    in_maps = []
    for c in range(NCORES):
        rs = slice(c * DPC, (c + 1) * DPC)
        w_slice = np.concatenate(
            [w_qkv[rs], w_qkv[D:2 * D][rs.start:rs.stop], w_qkv[2 * D:][rs.start:rs.stop]],
            axis=0,
        )  # [384, 1024]
        b_slice = np.concatenate(
            [b_qkv[rs], b_qkv[D:2 * D][rs.start:rs.stop], b_qkv[2 * D:][rs.start:rs.stop]]
        )[None, :]  # [1, 384]
        wt = np.ascontiguousarray(w_slice.T).astype(bf)           # [1024, 384]
        wq_t = np.ascontiguousarray(
            wt.reshape(KCH, 128, 3 * DPC).transpose(1, 0, 2)
        )  # [128, KCH, 384]
        m = {
            "xTt": Xt,
            "wt_qkv": wq_t,
            "b_qkv_s": np.ascontiguousarray(b_slice).astype(bf),
            "wt_proj": np.ascontiguousarray(w_proj[:, rs].T).astype(bf),
            "c_onesb": np.ones((1, 128), bf),
            "c_eye": eye.astype(bf),
        }
        if affine:
            gb = np.stack([q_gamma, q_beta, k_gamma, k_beta])  # [4, 64]
            m["c_gb"] = np.ascontiguousarray(
                np.broadcast_to(gb[None], (128, 4, HD)).astype(np.float32)
            )
        in_maps.append(m)
    return in_maps


_NC_CACHE = {}

LAST_RESULTS = None


def kernel(x, w_qkv, b_qkv, w_proj, b_proj, q_gamma, q_beta, k_gamma, k_beta,
           **unused):
    global LAST_RESULTS
    x = np.asarray(x, np.float32)
    w_qkv = np.asarray(w_qkv, np.float32)
    b_qkv = np.asarray(b_qkv, np.float32)
    w_proj = np.asarray(w_proj, np.float32)
    b_proj = np.asarray(b_proj, np.float32)
    q_gamma = np.asarray(q_gamma, np.float32)
    q_beta = np.asarray(q_beta, np.float32)
    k_gamma = np.asarray(k_gamma, np.float32)
    k_beta = np.asarray(k_beta, np.float32)

    B, S, _ = x.shape
    affine = not (
        np.all(q_gamma == 1) and np.all(k_gamma == 1)
        and np.all(q_beta == 0) and np.all(k_beta == 0)
    )
    key = (B, S, affine)
    if key not in _NC_CACHE:
        _NC_CACHE[key] = build_nc(B, S, affine)
    nc = _NC_CACHE[key]

    in_maps = make_in_maps(
        x, w_qkv, b_qkv, w_proj, b_proj, q_gamma, q_beta, k_gamma, k_beta, affine
    )
    trace = bool(int(os.environ.get("BASS_KERNEL_TRACE", "0")))
    res = run_bass_kernel_spmd(
        nc, in_maps, core_ids=list(range(NCORES)), trace=trace
    )
    LAST_RESULTS = res
    acc = np.zeros((D, B * S), np.float32)
    for r in res.results:
        acc += np.asarray(r["outp"], dtype=np.float32)
    out = acc.T + b_proj[None, :]
    return np.ascontiguousarray(out).reshape(B, S, D)


# revision 69
# speedup vs baseline: 1.0174x; 1.0174x over previous
"""Multi-head self-attention (B=2, S=2048, D=1024, H=16) on 8 TRN2 NeuronCores.

Tensor-parallel over heads: each core owns 2 heads. Accepts FULL inputs,
returns FULL output. Host pre-tiles x^T and slices per-head weights; each
core computes qkv -> per-head LayerNorm -> attention -> transposed partial
projection (over its 128 embed dims); host sums the 8 partials, transposes,
and adds the projection bias.

Perf notes (vs the first working version):
- softmax denominator: vO carries 64 ones-columns so the AV matmul
  broadcasts the denominator across partitions 64:128 for free; one DVE
  divide per head replaces the Ln -> PE-broadcast -> Exp(-x) -> mult chain.
- ACT only ever runs Exp/Ln/Copy (rstd = exp(-0.5*ln(var+eps))), all in
  one activation-table set -> no ACT_TABLE_LOAD switches.
- projection runs transposed (wp stationary, aT moving) and its PSUM is
  DMA'd straight to DRAM; bias is added on the host after the 8-way sum.
- PE instruction stream is software-pipelined (scores a step ahead of AV,
  prev-qc projection matmuls fill the qc-boundary epilogue gap).
"""

import os
import sys

import numpy as np

for _p in ("/opt/trn_rl_repo", "/root/.axon_site/_ro/trn_rl_repo"):
    if os.path.isdir(_p) and _p not in sys.path:
        sys.path.insert(0, _p)
        break

import concourse.bass as bass  # noqa: E402
import concourse.bacc as bacc  # noqa: E402
import concourse.tile as tile  # noqa: E402
from concourse import mybir  # noqa: E402
from concourse.bass_utils import run_bass_kernel_spmd  # noqa: E402

F32 = mybir.dt.float32
BF16 = mybir.dt.bfloat16
AF = mybir.ActivationFunctionType
ALU = mybir.AluOpType

NCORES = 8
D = 1024
H = 16
HD = 64
HPC = H // NCORES          # heads per core = 2
DPC = HPC * HD             # embed dims per core = 128
EPS = 1e-5


def build_nc(B, S, affine):
    """Build the SPMD Bass program for one core (same program, 8 cores)."""
    T = B * S                      # total token columns
    NTB = T // 128                 # 128-token blocks (32)
    NCH = T // 512                 # 512-token chunks (8)
    QC = S // 512                  # q-chunks per batch (4)
    KB = S // 128                  # k-blocks per batch (16)
    KCH = D // 128                 # contraction chunks (8)
    SCALE = 1.0 / np.sqrt(HD)

    nc = bacc.Bacc(
        "TRN2",
        target_bir_lowering=False,
        debug=False,
        enable_asserts=True,
        num_devices=NCORES,
    )

    # host-pretiled x^T: [128, NCH, KCH, 512] (p, chunk, k, tok)
    xT = nc.dram_tensor("xTt", [128, NCH, KCH, 512], BF16, kind="ExternalInput").ap()
    wq = nc.dram_tensor("wt_qkv", [128, KCH, 3 * DPC], BF16, kind="ExternalInput").ap()
    bq = nc.dram_tensor("b_qkv_s", [1, 3 * DPC], BF16, kind="ExternalInput").ap()
    wp = nc.dram_tensor("wt_proj", [DPC, D], BF16, kind="ExternalInput").ap()
    onesb = nc.dram_tensor("c_onesb", [1, 128], BF16, kind="ExternalInput").ap()
    eye = nc.dram_tensor("c_eye", [128, 128], BF16, kind="ExternalInput").ap()
    if affine:
        gb = nc.dram_tensor("c_gb", [128, 4, HD], F32, kind="ExternalInput").ap()
    # transposed partial projection output: [D, T] (bf16 partials; host sums)
    outp = nc.dram_tensor("outp", [D, T], BF16, kind="ExternalOutput").ap()

    from contextlib import ExitStack

    with tile.TileContext(nc) as tc, ExitStack() as stack:
        const = stack.enter_context(tc.tile_pool(name="const", bufs=1))
        persist = stack.enter_context(tc.tile_pool(name="persist", bufs=1))

        # startup DMAs spread across queue rings so the first matmul's
        # prerequisites (bq, onesb, wq k0, xt k0) land as early as possible
        bq_sb = const.tile([1, 3 * DPC], BF16, tag="bq")
        nc.gpsimd.dma_start(out=bq_sb, in_=bq)
        onesb_sb = const.tile([1, 128], BF16, tag="onesb")
        nc.gpsimd.dma_start(out=onesb_sb, in_=onesb)
        wq_sb = const.tile([128, KCH, 3 * DPC], BF16, tag="wq")
        for _k in range(KCH):
            nc.scalar.dma_start(out=wq_sb[:, _k, :], in_=wq[:, _k, :])
        wp_sb = const.tile([DPC, D], BF16, tag="wp")
        nc.gpsimd.dma_start(out=wp_sb, in_=wp)
        eye_sb = const.tile([128, 128], BF16, tag="eye")
        nc.gpsimd.dma_start(out=eye_sb, in_=eye)
        eps_sb = const.tile([128, 1], F32, tag="eps")
        nc.vector.memset(eps_sb, EPS)

        if affine:
            gb_sb = const.tile([128, 4, HD], F32, tag="gb")
            nc.sync.dma_start(out=gb_sb, in_=gb)

        # persistent intermediates
        qT = persist.tile([128, T], BF16, tag="qT")     # [2h*64, tok] LN'd q^T
        kT = persist.tile([128, T], BF16, tag="kT")
        # 64 ones-cols + v per head: AV matmul then yields softmax denom
        # broadcast over partitions 0:64 at zero extra PE cost (ones first so
        # the denominator lands at partition base 0, which the custom-DVE
        # reciprocal requires)
        vO = persist.tile([128, HPC, NTB, 128], BF16, tag="vO")
        aT = persist.tile([128, T], BF16, tag="aT")     # attention out^T
        nc.gpsimd.memset(vO[:, :, :, 0:HD], 1.0)

        # stage1 outlives phase 1: the last blocks' transposes are deferred
        # into phase 2 (SBUF-only pool, no PSUM cost)
        stage1 = stack.enter_context(tc.tile_pool(name="stage1", bufs=6))
        pending = []  # (qn, kn, tb)

        # ---------------- Phase 1: qkv + LayerNorm + transpose ----------
        with (
            tc.tile_pool(name="xt", bufs=2) as xt_pool,
            tc.tile_pool(name="qkv_ps", bufs=6, space="PSUM") as qkv_ps,
            tc.tile_pool(name="t_ps", bufs=2, space="PSUM") as t_ps,
            tc.tile_pool(name="stats", bufs=6) as stats_pool,
        ):
            # software pipeline: transposes/copies of block tb are emitted
            # two blocks later in the matmul stream so the PE never waits on
            # the LayerNorm chain.
            def flush_one():
                qn, kn, tb = pending.pop(0)
                tp = t_ps.tile([128, 256], BF16, tag="tp")
                nc.tensor.transpose(tp[:, 0:128], qn, eye_sb)
                nc.tensor.transpose(tp[:, 128:256], kn, eye_sb)
                ts = slice(tb * 128, (tb + 1) * 128)
                nc.vector.tensor_copy(out=qT[:, ts], in_=tp[:, 0:128])
                nc.scalar.copy(out=kT[:, ts], in_=tp[:, 128:256])

            # stage-B of the LN chain runs one block behind stage-A so no
            # engine ever waits on a fresh cross-engine result
            chain = []  # (qk, ps, mv, tb)

            def ln_finish():
                qk, ps, mv, tb = chain.pop(0)
                rstd = stats_pool.tile([128, 4], F32, tag="rstd")
                nc.scalar.activation(
                    out=rstd, in_=mv[:, :, 1], func=AF.Sqrt, bias=eps_sb
                )
                nc.vector.reciprocal(out=rstd, in_=rstd)
                # nmr = -mean*rstd lets ACT apply LN as x*rstd + nmr
                nmr = stats_pool.tile([128, 4], F32, tag="nmr")
                nc.vector.scalar_tensor_tensor(
                    out=nmr, in0=mv[:, :, 0], scalar=-1.0, in1=rstd,
                    op0=ALU.mult, op1=ALU.mult,
                )
                qn = stage1.tile([128, 128], BF16, tag="qn")
                kn = stage1.tile([128, 128], BF16, tag="kn")
                for g in range(4):
                    dst = qn if g < 2 else kn
                    dsl = dst[:, (g % 2) * HD : (g % 2 + 1) * HD]
                    if g < 2:
                        nc.vector.tensor_scalar(
                            out=dsl,
                            in0=qk[:, g, :],
                            scalar1=mv[:, g, 0:1],
                            scalar2=rstd[:, g : g + 1],
                            op0=ALU.subtract,
                            op1=ALU.mult,
                        )
                    else:
                        nc.scalar.activation(
                            out=dsl,
                            in_=qk[:, g, :],
                            func=AF.Identity,
                            scale=rstd[:, g : g + 1],
                            bias=nmr[:, g : g + 1],
                        )
                    if affine:
                        nc.vector.tensor_mul(dsl, dsl, gb_sb[:, 2 * (g // 2), :])
                        nc.vector.tensor_add(
                            dsl, dsl, gb_sb[:, 2 * (g // 2) + 1, :]
                        )
                # v into vO (ones-cols already memset)
                nc.scalar.copy(
                    out=vO[:, 0, tb, HD:128],
                    in_=ps[:, 2 * DPC : 2 * DPC + HD],
                )
                nc.scalar.copy(
                    out=vO[:, 1, tb, HD:128],
                    in_=ps[:, 2 * DPC + HD : 2 * DPC + 2 * HD],
                )
                pending.append((qn, kn, tb))

            for n in range(NCH):
                xt = xt_pool.tile([128, KCH, 512], BF16, tag="xt")
                if n == 0:
                    # split the first chunk per k so the first matmuls can
                    # start after 1/8 of the transfer
                    for k in range(KCH):
                        nc.sync.dma_start(out=xt[:, k, :], in_=xT[:, 0, k, :])
                else:
                    nc.sync.dma_start(out=xt, in_=xT[:, n])
                for tbl in range(4):
                    tb = n * 4 + tbl
                    ps = qkv_ps.tile([128, 3 * DPC], F32, tag="ps")
                    # bias first on even blocks, last on odd blocks: the K=1
                    # bias matmuls of adjacent blocks pair up so the PE pays
                    # one K=1<->K=128 transition per block instead of two
                    bias_first = tb % 2 == 0
                    if bias_first:
                        nc.tensor.matmul(
                            ps, lhsT=onesb_sb, rhs=bq_sb, start=True, stop=False
                        )
                    for k in range(KCH):
                        if k == 4 and len(pending) >= 3:
                            flush_one()  # transposes, 3+ blocks behind
                        nc.tensor.matmul(
                            ps,
                            lhsT=xt[:, k, tbl * 128 : (tbl + 1) * 128],
                            rhs=wq_sb[:, k, :],
                            start=(not bias_first and k == 0),
                            stop=(bias_first and k == KCH - 1),
                        )
                    if not bias_first:
                        nc.tensor.matmul(
                            ps, lhsT=onesb_sb, rhs=bq_sb, start=False, stop=True
                        )
                    # LayerNorm stats over each head's 64 dims of q and k
                    qk = ps[:, 0 : 2 * DPC].rearrange("p (g d) -> p g d", d=HD)
                    st = stats_pool.tile([128, 4, 6], F32, tag="st")
                    mv = stats_pool.tile([128, 4, 2], F32, tag="mv")
                    for g in range(4):
                        nc.vector.bn_stats(out=st[:, g, :], in_=qk[:, g, :])
                        nc.vector.bn_aggr(out=mv[:, g, :], in_=st[:, g, :])
                    chain.append((qk, ps, mv, tb))
                    if len(chain) >= 2:
                        ln_finish()  # block tb-1's sqrt/apply/v
            while chain:
                ln_finish()
            # leftover `pending` transposes are deferred into phase 2: the
            # first scores don't need them, and they fill the exp warmup

        # ---------------- Phase 2: attention + fused projection ---------
        with (
            tc.tile_pool(name="sc_ps", bufs=2, space="PSUM") as sc_ps,
            tc.tile_pool(name="o_ps", bufs=3, space="PSUM") as o_ps,
            tc.tile_pool(name="p_ps", bufs=1, space="PSUM") as p_ps,
            tc.tile_pool(name="exps", bufs=4) as exps,
            tc.tile_pool(name="ostage", bufs=3) as ostage,
        ):
            # proj matmuls for q-chunk (b, qc) are emitted inside the next
            # q-chunk's kb loop (aT is ready by then)
            proj_ctr = [0]

            def emit_proj_one(b, qc, nb, drain=False):
                pcols = slice(b * S + qc * 512, b * S + (qc + 1) * 512)
                alt = proj_ctr[0] % 2 == 0
                pps = p_ps.tile([128, 512], F32, tag="pps", name="pps")
                nc.tensor.matmul(
                    pps,
                    lhsT=wp_sb[:, nb * 128 : (nb + 1) * 128],
                    rhs=aT[:, pcols],
                    start=True,
                    stop=True,
                )
                ob = ostage.tile([128, 512], BF16, tag="ob")
                if drain and alt:
                    # ACT is idle after the last exp: split the evictions
                    nc.scalar.copy(out=ob, in_=pps)
                else:
                    nc.vector.tensor_copy(out=ob, in_=pps)
                dq = nc.gpsimd if alt else nc.sync
                proj_ctr[0] += 1
                dq.dma_start(
                    out=outp[nb * 128 : (nb + 1) * 128, pcols], in_=ob
                )

            def emit_scores(b, qc, kb):
                cols = slice(b * S + qc * 512, b * S + (qc + 1) * 512)
                gkb = b * KB + kb
                ks = slice(gkb * 128, (gkb + 1) * 128)
                scp = sc_ps.tile([128, HPC, 512], F32, tag="s", name="scp")
                for h in range(HPC):
                    hp = slice(h * HD, (h + 1) * HD)
                    nc.tensor.matmul(
                        scp[:, h, :],
                        lhsT=kT[hp, ks],
                        rhs=qT[hp, cols],
                        start=True,
                        stop=True,
                    )
                ex = exps.tile([128, HPC, 512], BF16, tag="ex", name="ex")
                nc.scalar.activation(out=ex, in_=scp, func=AF.Exp, scale=SCALE)
                return ex

            proj_jobs = []  # flat (b, qc, nb) list, aT already readable

            # one flat software pipeline over every (b, qc, kb) step: scores
            # run two steps ahead of AV, across q-chunk boundaries, so
            # neither the PE nor ACT ever waits at a boundary
            steps = [
                (b, qc, kb)
                for b in range(B)
                for qc in range(QC)
                for kb in range(KB)
            ]
            exq = [
                emit_scores(*steps[0]),
                emit_scores(*steps[1]),
                emit_scores(*steps[2]),
            ]
            # phase-1 leftovers: last blocks' transposes, hidden under the
            # first exps (their kT/qT regions aren't read until batch 1)
            while pending:
                qn, kn, tb = pending.pop(0)
                tp = p_ps.tile([128, 512], F32, tag="pps", name="tpd")
                tpb = tp.bitcast(BF16)[:, 0:256]
                nc.tensor.transpose(tpb[:, 0:128], qn, eye_sb)
                nc.tensor.transpose(tpb[:, 128:256], kn, eye_sb)
                ts2 = slice(tb * 128, (tb + 1) * 128)
                nc.vector.tensor_copy(out=qT[:, ts2], in_=tpb[:, 0:128])
                nc.vector.tensor_copy(out=kT[:, ts2], in_=tpb[:, 128:256])
            oom = None
            for i, (b, qc, kb) in enumerate(steps):
                cols = slice(b * S + qc * 512, b * S + (qc + 1) * 512)
                if kb == 0:
                    # per-head oom tiles from a 3-deep pool: this q-chunk's
                    # first AV doesn't wait for the previous one's drain
                    oom = [
                        o_ps.tile([128, 512], F32, tag="o", name=f"oom{h}")
                        for h in range(HPC)
                    ]
                if i + 3 < len(steps):
                    exq.append(emit_scores(*steps[i + 3]))
                # AV for kb
                cur_ex = exq.pop(0)
                gkb = b * KB + kb
                for h in range(HPC):
                    nc.tensor.matmul(
                        oom[h],
                        lhsT=vO[:, h, gkb, :],
                        rhs=cur_ex[:, h, :],
                        start=(kb == 0),
                        stop=(kb == KB - 1),
                    )
                # prev-qc projection matmuls keep the PE dense while this
                # q-chunk's exps run; start at kb=6 so their DVE evictions
                # don't queue behind the previous epilogue's DVE chain
                if proj_jobs and 6 <= kb < 14:
                    emit_proj_one(*proj_jobs.pop(0))
                if kb == KB - 1:
                    # epilogue: denominator is broadcast over rows 0:64 of
                    # oom; stage it for the custom-DVE reciprocal, multiply
                    # the AV half (rows 64:128) straight from PSUM. The
                    # 3-deep oom pool hides this chain from the next q-chunk.
                    last = i == len(steps) - 1
                    dnms = []
                    for h in range(HPC):
                        dnm = ostage.tile([HD, 512], F32, tag=f"dnm{h}")
                        # on the very last chunk ACT stages head 1's denom in
                        # parallel to shorten the serial DVE tail chain
                        if last and h == 1:
                            nc.scalar.copy(out=dnm, in_=oom[h][0:HD, :])
                        else:
                            nc.vector.tensor_copy(out=dnm, in_=oom[h][0:HD, :])
                        dnms.append(dnm)
                    for h in range(HPC):
                        rcp = ostage.tile([HD, 512], F32, tag=f"rcp{h}")
                        nc.vector.reciprocal_approx_fast(out=rcp, in_=dnms[h])
                        nc.vector.tensor_mul(
                            aT[h * HD : (h + 1) * HD, cols],
                            oom[h][HD:128, :],
                            rcp,
                        )
                    proj_jobs.extend((b, qc, nb) for nb in range(KCH))
            # drain remaining projections
            for job in proj_jobs:
                emit_proj_one(*job, drain=True)

    nc.compile()
    return nc


def make_in_maps(x, w_qkv, b_qkv, w_proj, b_proj, q_gamma, q_beta, k_gamma, k_beta,
                 affine):
    import ml_dtypes
    bf = ml_dtypes.bfloat16
    B, S, _ = x.shape
    T = B * S
    NCH = T // 512
    KCH = D // 128
    # x^T tiled: X[p, n, c, t] = x2[c*128+p, n*512+t] where x2 = x.reshape(T,D).T
    x2 = np.ascontiguousarray(x.reshape(T, D).T.astype(bf))       # [D, T]
    Xt = np.ascontiguousarray(
        x2.reshape(KCH, 128, NCH, 512).transpose(1, 2, 0, 3)
    )  # [128, NCH, KCH, 512]
    eye = np.eye(128, dtype=np.float32)
    in_maps = []
    for c in range(NCORES):
        rs = slice(c * DPC, (c + 1) * DPC)
        w_slice = np.concatenate(
            [w_qkv[rs], w_qkv[D:2 * D][rs.start:rs.stop], w_qkv[2 * D:][rs.start:rs.stop]],
            axis=0,
        )  # [384, 1024]
        b_slice = np.concatenate(
            [b_qkv[rs], b_qkv[D:2 * D][rs.start:rs.stop], b_qkv[2 * D:][rs.start:rs.stop]]
        )[None, :]  # [1, 384]
        wt = np.ascontiguousarray(w_slice.T).astype(bf)           # [1024, 384]
        wq_t = np.ascontiguousarray(
            wt.reshape(KCH, 128, 3 * DPC).transpose(1, 0, 2)
        )  # [128, KCH, 384]
        m = {
            "xTt": Xt,
            "wt_qkv": wq_t,
            "b_qkv_s": np.ascontiguousarray(b_slice).astype(bf),
            "wt_proj": np.ascontiguousarray(w_proj[:, rs].T).astype(bf),
            "c_onesb": np.ones((1, 128), bf),
            "c_eye": eye.astype(bf),
        }
        if affine:
            gb = np.stack([q_gamma, q_beta, k_gamma, k_beta])  # [4, 64]
            m["c_gb"] = np.ascontiguousarray(
                np.broadcast_to(gb[None], (128, 4, HD)).astype(np.float32)
            )
        in_maps.append(m)
    return in_maps


_NC_CACHE = {}

LAST_RESULTS = None


def kernel(x, w_qkv, b_qkv, w_proj, b_proj, q_gamma, q_beta, k_gamma, k_beta,
           **unused):
    global LAST_RESULTS
    x = np.asarray(x, np.float32)
    w_qkv = np.asarray(w_qkv, np.float32)
    b_qkv = np.asarray(b_qkv, np.float32)
    w_proj = np.asarray(w_proj, np.float32)
    b_proj = np.asarray(b_proj, np.float32)
    q_gamma = np.asarray(q_gamma, np.float32)
    q_beta = np.asarray(q_beta, np.float32)
    k_gamma = np.asarray(k_gamma, np.float32)
    k_beta = np.asarray(k_beta, np.float32)

    B, S, _ = x.shape
    affine = not (
        np.all(q_gamma == 1) and np.all(k_gamma == 1)
        and np.all(q_beta == 0) and np.all(k_beta == 0)
    )
    key = (B, S, affine)
    if key not in _NC_CACHE:
        _NC_CACHE[key] = build_nc(B, S, affine)
    nc = _NC_CACHE[key]

    in_maps = make_in_maps(
        x, w_qkv, b_qkv, w_proj, b_proj, q_gamma, q_beta, k_gamma, k_beta, affine
    )
    trace = bool(int(os.environ.get("BASS_KERNEL_TRACE", "0")))
    res = run_bass_kernel_spmd(
        nc, in_maps, core_ids=list(range(NCORES)), trace=trace
    )
    LAST_RESULTS = res
    acc = np.zeros((D, B * S), np.float32)
    for r in res.results:
        acc += np.asarray(r["outp"], dtype=np.float32)
    out = acc.T + b_proj[None, :]
    return np.ascontiguousarray(out).reshape(B, S, D)
